# revision 63
# baseline (speedup 1.0000x reference)
"""Trainium2 Bass kernel for nn_Attention_Rel_Scl (B=4,S=1024,E=1024,H=16).

Sharding: 8 cores = (batch b, head-half hg). Core c = 2*b + hg computes, for
batch b, heads 8*hg..8*hg+7 over the FULL sequence:
  out[:, 512*hg:512*hg+512] = LN-half of
      concat_h[ (softmax(q k^T / 32) + relbias_h) @ v_h ]
Zero-duplication split (each projection column computed once fleet-wide).
LayerNorm needs full-E row stats, so core pairs (2b, 2b+1) exchange per-row
partial bn_stats via a tiny AllGather (12 KB), per 512-query chunk; chunk 0's
collective hides under chunk 1's compute.

Optimizations vs the original baseline (196.8us -> 136.3us cost model):
  - PV and rel-bias matmuls flipped to [q-part, d-free] outputs:
      lhsT = E-block / Toeplitz-bias-block [128 kv, 128 q], rhs = V [128, 65]
    Full 128-wide contraction AND <=65-row moving dim -> half the PE rows of
    the old [65, 512] layout, and no PE transposes / staging copies at all.
    Both accumulate into one [128, 129] PSUM tile (cols 0:65 = PV|Z, 65:129 =
    bias term); DVE combines: out = PV * (1/Z) + biasV straight from PSUM.
  - all matmul inputs bf16 (incl. x^T and W) -> halved DMA bytes + SBUF;
    y output returned bf16 (host casts back) -> halved final writeback.
  - host pre-tiled DRAM layouts so every big DMA moves >=1KB contiguous
    elements (avoids the <512B 2x DMA latency penalty); k-major first
    projection consumes streamed xT chunks as they land.
  - software-pipelined schedule: scores for (m, chunk) run 1-3 rounds ahead
    of their PV consumers; chunk-1 score blocks for m=0,1 are precomputed
    inside chunk 0 and V trails one round, so the PE always has matmul work
    while the Activation engine grinds the exp stream (Act is the secondary
    bottleneck; PSUM allows only 2 score tiles = 2 exps of lookahead).
  - LN stats split into a 256-wide partial (heads 0-3, computed as soon as
    pv(m=1) lands) and a 256-wide tail partial per q-block inside the last
    head-pair; both partials are locally bn_aggr'd to (mean, var) so the
    AllGather moves only 8 floats/row (collective cost is 15us + bytes/40),
    and fires ~1us after the last matmul. Cross-core merge after the
    exchange: mean=(mA+mB)/2, var=(vA+vB)/2+(mA-mB)^2/4, batched over all
    four q-blocks in 6 small DVE ops.
  - chunk-0 LN + y writeback hide under the final collective; final-chunk
    LN alternates DVE tensor_scalar and Act Identity(scale,bias) per
    q-block, with paired 2-block y DMAs. The last head-pair's 1/Z scaling
    runs on Act (idle by then) to shorten the DVE chain into the collective.
"""

import sys

if "/opt/trn_rl_repo" not in sys.path:
    sys.path.insert(0, "/opt/trn_rl_repo")

import numpy as np
import ml_dtypes

import concourse.bass as bass
import concourse.mybir as mybir
import concourse.tile as tile
from concourse import bacc
from concourse.bass_utils import run_bass_kernel_spmd

B, S, E, H = 4, 1024, 1024, 16
D = E // H          # 64
HC = H // 2         # 8 heads per core
EC = HC * D         # 512 output columns per core
NK = E // 128       # 8 contraction blocks
NQB = S // 128      # 8 query blocks (full sequence per core)
SCALE = float(E) ** -0.5
LN_EPS = 1e-5
TW = 1920           # Toeplitz window width

F32 = mybir.dt.float32
BF16 = mybir.dt.bfloat16

_cache = {}


def _build_nc():
    nc = bacc.Bacc("TRN2", target_bir_lowering=False, debug=False, num_devices=8)

    # host-pretiled inputs (see kernel() for layouts)
    xT = nc.dram_tensor("xT", [128, NK, S], BF16, kind="ExternalInput").ap()
    wkq = nc.dram_tensor("wkq", [128, 8, NK, 128], BF16, kind="ExternalInput").ap()
    wv = nc.dram_tensor("wv", [128, NK, EC], BF16, kind="ExternalInput").ap()
    tbl = nc.dram_tensor("tbl", [HC, 128, TW], BF16, kind="ExternalInput").ap()
    y = nc.dram_tensor("y", [S, EC], BF16, kind="ExternalOutput").ap()
    cc = [
        (
            nc.dram_tensor(f"cc_in{i}", [128, 8], F32).ap(),
            nc.dram_tensor(f"cc_out{i}", [256, 8], F32).ap(),
        )
        for i in range(2)
    ]

    with tile.TileContext(nc) as tc:
        _emit(nc, tc, xT, wkq, wv, tbl, y, cc)
    nc.finalize()
    return nc


def _emit(nc, tc, xT, wkq, wv, tbl, y, cc):
    import contextlib

    ctx = contextlib.ExitStack()
    with ctx:
        singles = ctx.enter_context(tc.tile_pool(name="singles", bufs=1))
        epool = ctx.enter_context(tc.tile_pool(name="epool", bufs=8))
        small = ctx.enter_context(tc.tile_pool(name="small", bufs=8))
        pst = ctx.enter_context(tc.tile_pool(name="pst", bufs=2, space="PSUM"))
        ppv = ctx.enter_context(tc.tile_pool(name="ppv", bufs=4, space="PSUM"))

        # ---- resident SBUF tensors --------------------------------------
        # stream wk0/wq0 k-halves and xT chunks so the k-major first
        # projection starts ~3us in and never outruns the data
        wkq_sb = singles.tile([128, 8, NK, 128], BF16)   # j: wk m=0..3, wq m=0..3
        xT_sb = singles.tile([128, NK, S], BF16)         # 2 MB
        nc.scalar.dma_start(out=wkq_sb[:, 0, 0:4, :], in_=wkq[:, 0, 0:4, :])
        nc.sync.dma_start(out=xT_sb[:, 0, 0:512], in_=xT[:, 0, 0:512])
        nc.scalar.dma_start(out=wkq_sb[:, 4, 0:4, :], in_=wkq[:, 4, 0:4, :])
        nc.sync.dma_start(out=xT_sb[:, 0, 512:1024], in_=xT[:, 0, 512:1024])
        nc.sync.dma_start(out=xT_sb[:, 1:2, :], in_=xT[:, 1:2, :])
        nc.scalar.dma_start(out=wkq_sb[:, 0, 4:8, :], in_=wkq[:, 0, 4:8, :])
        nc.scalar.dma_start(out=wkq_sb[:, 4, 4:8, :], in_=wkq[:, 4, 4:8, :])
        for k in range(1, NK // 2):
            nc.sync.dma_start(
                out=xT_sb[:, 2 * k:2 * k + 2, :], in_=xT[:, 2 * k:2 * k + 2, :]
            )
        for m in range(1, 4):
            for j in range(2):
                nc.scalar.dma_start(
                    out=wkq_sb[:, 4 * j + m, :, :], in_=wkq[:, 4 * j + m, :, :]
                )
        wv_sb = singles.tile([128, NK, EC], BF16)
        nc.scalar.dma_start(out=wv_sb, in_=wv)
        tbl_sb = singles.tile([128, HC, TW], BF16)       # Toeplitz bias windows
        for h in range(HC):
            nc.scalar.dma_start(out=tbl_sb[:, h, :], in_=tbl[h])

        eps_t = singles.tile([128, 1], F32)
        nc.vector.memset(eps_t, LN_EPS)
        negones = singles.tile([128, 1], F32)
        nc.vector.memset(negones, -1.0)
        half_t = singles.tile([128, 1], F32)
        nc.vector.memset(half_t, 0.5)
        quart_t = singles.tile([128, 1], F32)
        nc.vector.memset(quart_t, 0.25)
        y_sb = singles.tile([128, NQB, EC], BF16)   # normalized bf16 output

        # V natural layout + ones column, bf16: [128 s-in-block, sb, head, 65]
        v_sb = singles.tile([128, NK, HC, D + 1], BF16)
        nc.vector.memset(v_sb[:, :, :, D:D + 1], 1.0)

        out_sb = singles.tile([128, NQB, EC], F32)       # 2 MB
        kts = [singles.tile([128, S], BF16, name=f"kt{m}") for m in range(4)]
        qts = [singles.tile([128, S], BF16, name=f"qt{m}") for m in range(4)]

        # ---- emitters ----------------------------------------------------
        def emit_kq0():
            # k-major variant for the very first projection: all four PSUM
            # groups open at once so PE can consume xT k-chunks as they land.
            pss = [ppv.tile([128, 512], F32, tag="pv", name=f"pkq0_{i}")
                   for i in range(4)]
            for k in range(NK):
                for i, (dj, n) in enumerate(((0, 0), (0, 1), (4, 0), (4, 1))):
                    nc.tensor.matmul(
                        pss[i], lhsT=wkq_sb[:, dj, k, :],
                        rhs=xT_sb[:, k, 512 * n:512 * (n + 1)],
                        start=(k == 0), stop=(k == NK - 1),
                    )
            for i, (dj, n) in enumerate(((0, 0), (0, 1), (4, 0), (4, 1))):
                dst = kts[0] if dj == 0 else qts[0]
                nc.vector.tensor_copy(
                    out=dst[:, 512 * n:512 * (n + 1)], in_=pss[i])

        def emit_kq(m):
            # kts[m][e', s] (e' = head-pair m's 128 cols), same for qts
            for dj, dst in ((0, kts[m]), (4, qts[m])):
                for n in range(2):
                    ps = ppv.tile([128, 512], F32, tag="pv", name=f"pkq{m}{dj}{n}")
                    for k in range(NK):
                        nc.tensor.matmul(
                            ps, lhsT=wkq_sb[:, dj + m, k, :],
                            rhs=xT_sb[:, k, 512 * n:512 * (n + 1)],
                            start=(k == 0), stop=(k == NK - 1),
                        )
                    nc.vector.tensor_copy(out=dst[:, 512 * n:512 * (n + 1)], in_=ps)

        def gen_v():
            for m in range(NK):  # s block
                ps = ppv.tile([128, 512], F32, tag="pv", name=f"psv{m}")
                for k in range(NK):
                    nc.tensor.matmul(
                        ps, lhsT=xT_sb[:, k, 128 * m:128 * (m + 1)],
                        rhs=wv_sb[:, k, :],
                        start=(k == 0), stop=(k == NK - 1),
                    )
                nc.vector.tensor_copy(
                    out=v_sb[:, m, :, 0:D],
                    in_=ps.rearrange("p (h d) -> p h d", d=D),
                )
                yield

        def gen_scores_exp(m, qch, e_pair):
            # E[kv, q] for heads 2m (hl=0) / 2m+1 (hl=1), query chunk qch.
            # Generator: one step per k-block pair (4 matmuls + 2 exps).
            kt, qt = kts[m], qts[m]
            q0 = 512 * qch
            for kp in range(NK // 2):
                st = [
                    pst.tile([128, 1024], F32, tag="st", name=f"st{m}{qch}{kp}{hl}")
                    for hl in range(2)
                ]
                for kh in range(2):
                    kb = 2 * kp + kh
                    for hl in range(2):
                        nc.tensor.matmul(
                            st[hl][:, 512 * kh:512 * (kh + 1)],
                            lhsT=kt[64 * hl:64 * hl + D, 128 * kb:128 * (kb + 1)],
                            rhs=qt[64 * hl:64 * hl + D, q0:q0 + 512],
                            start=True, stop=True,
                        )
                for hl in range(2):
                    nc.scalar.activation(
                        out=e_pair[hl].rearrange("p a b -> p (a b)")[
                            :, 1024 * kp:1024 * (kp + 1)],
                        in_=st[hl],
                        func=mybir.ActivationFunctionType.Exp,
                        scale=SCALE,
                    )
                yield

        def new_epair(m, qch):
            return [
                epool.tile([128, NK, 512], BF16, tag="eh", name=f"e{m}{qch}{hl}")
                for hl in range(2)
            ]

        def pv_one(m, qch, e_pair, hl, ql, stats, act_combine=False,
                   pstile=None, c0=0):
            # flipped layout: out[q, d] for one (head, 128-q block)
            h = 2 * m + hl          # head index within the core's half
            qb = 4 * qch + ql
            ps = pstile[:, c0:c0 + 129]
            for kb in range(NK):
                nc.tensor.matmul(
                    ps[:, 0:D + 1],
                    lhsT=e_pair[hl][:, kb, 128 * ql:128 * (ql + 1)],
                    rhs=v_sb[:, kb, h, :],
                    start=(kb == 0), stop=(kb == NK - 1),
                )
            for kb in range(NK):
                off = 128 * (qb - kb) + 896
                nc.tensor.matmul(
                    ps[:, D + 1:2 * D + 1],
                    lhsT=tbl_sb[:, h, off:off + 128],
                    rhs=v_sb[:, kb, h, 0:D],
                    start=(kb == 0), stop=(kb == NK - 1),
                )
            rz = small.tile([128, 1], F32, tag="rz", name=f"rz{h}{qb}")
            nc.vector.reciprocal(rz, ps[:, D:D + 1])
            dst = out_sb[:, qb, D * h:D * (h + 1)]
            if act_combine:
                nc.scalar.activation(
                    out=dst, in_=ps[:, 0:D],
                    func=mybir.ActivationFunctionType.Copy, scale=rz,
                )
                nc.vector.tensor_add(out=dst, in0=dst,
                                     in1=ps[:, D + 1:2 * D + 1])
            else:
                nc.vector.tensor_scalar(
                    out=dst, in0=ps[:, 0:D], scalar1=rz, scalar2=None,
                    op0=mybir.AluOpType.mult,
                )
                nc.vector.tensor_add(out=dst, in0=dst,
                                     in1=ps[:, D + 1:2 * D + 1])

        def gen_pv(m, qch, e_pair, stats=None, tail=False, act=False):
            # Generator: one step per (head, q-block); 8 steps.
            # stats + tail=False (m==1): q-block-major, emit the 256-wide
            #   heads-0..3 partial bn_stats as each q-block's first 4 heads
            #   complete (spread across steps so PV PSUM drains stay tight).
            # stats + tail=True (m==3): q-block-major, 256-wide heads-4..7
            #   partial per q-block + local bn_aggr; 1/Z scale on the Act
            #   engine so the DVE chain off the last matmul is short.
            if stats is None:
                steps = [(hl, ql) for hl in range(2) for ql in range(4)]
            else:
                steps = [(hl, ql) for ql in range(4) for hl in range(2)]
            for i, (hl, ql) in enumerate(steps):
                pstile = ppv.tile([128, 129], F32, tag="pv",
                                  name=f"pv{m}{qch}{i}")
                pv_one(m, qch, e_pair, hl, ql, None, act_combine=act,
                       pstile=pstile, c0=0)
                if stats is not None and hl == 1:
                    if tail:
                        nc.vector.bn_stats(
                            out=stats[:, ql, 1, :],
                            in_=out_sb[:, 4 * qch + ql, 4 * D:8 * D])
                        nc.vector.bn_aggr(out=mvloc[qch][:, ql, :],
                                          in_=stats[:, ql, :, :])
                    else:
                        nc.vector.bn_stats(
                            out=stats[:, ql, 0, :],
                            in_=out_sb[:, 4 * qch + ql, 0:4 * D])
                yield

        def interleave(sc_gen, pv_gen, ratio=2):
            # one score kp-step (4 mm), `ratio` pv steps per round
            for _ in range(4):
                next(sc_gen)
                for _ in range(ratio):
                    next(pv_gen, None)
            for _ in pv_gen:
                pass

        def emit_cc(qch, stats_sb):
            cc_in, cc_out = cc[qch]
            nc.sync.dma_start(out=cc_in,
                              in_=mvloc[qch].rearrange("p a b -> p (a b)"))
            nc.gpsimd.collective_compute(
                kind="AllGather",
                op=mybir.AluOpType.bypass,
                replica_groups=[[0, 1], [2, 3], [4, 5], [6, 7]],
                ins=[cc_in], outs=[cc_out],
            )
            # allst[:, r, ql, :] = pair-core r's (mean, var) for q-block ql
            allst = small.tile([128, 2, 4, 2], F32, tag="allst",
                               name=f"al{qch}")
            nc.sync.dma_start(
                out=allst,
                in_=bass.AP(tensor=cc_out.tensor, offset=cc_out.offset,
                            ap=[[8, 128], [8 * 128, 2], [1, 8]]),
            )
            # merge equal-count halves, all 4 q-blocks batched:
            #   mean = (mA+mB)/2 ; var = (vA+vB)/2 + (mA-mB)^2/4
            mvm = small.tile([128, 4, 2], F32, tag="mvm", name=f"mvm{qch}")
            nc.vector.tensor_add(out=mvm, in0=allst[:, 0, :, :],
                                 in1=allst[:, 1, :, :])
            nc.vector.tensor_scalar(
                out=mvm, in0=mvm, scalar1=half_t, scalar2=None,
                op0=mybir.AluOpType.mult)
            dd = small.tile([128, 4], F32, tag="dd", name=f"dd{qch}")
            nc.vector.tensor_sub(out=dd, in0=allst[:, 0, :, 0],
                                 in1=allst[:, 1, :, 0])
            nc.vector.tensor_mul(out=dd, in0=dd, in1=dd)
            nc.vector.tensor_scalar(
                out=dd, in0=dd, scalar1=quart_t, scalar2=None,
                op0=mybir.AluOpType.mult)
            nc.vector.tensor_add(out=mvm[:, :, 1], in0=mvm[:, :, 1], in1=dd)
            return mvm

        def emit_ln_chunk(qch, mvm, split_y):
            # per-block rstd chain, normalize alternating Act/DVE
            for ql in range(4):
                qb = 4 * qch + ql
                rstd = small.tile([128, 1], F32, tag="rstd", name=f"rs{qb}")
                nc.scalar.activation(
                    out=rstd, in_=mvm[:, ql, 1:2],
                    func=mybir.ActivationFunctionType.Sqrt,
                    bias=eps_t, scale=1.0,
                )
                nc.vector.reciprocal(rstd, rstd)
                if ql % 2 == 0:
                    nb = small.tile([128, 1], F32, tag="nb", name=f"nb{qb}")
                    nc.vector.tensor_scalar(
                        out=nb, in0=mvm[:, ql, 0:1], scalar1=rstd,
                        scalar2=negones,
                        op0=mybir.AluOpType.mult, op1=mybir.AluOpType.mult,
                    )
                    nc.scalar.activation(
                        out=y_sb[:, qb, :], in_=out_sb[:, qb, :],
                        func=mybir.ActivationFunctionType.Identity,
                        scale=rstd, bias=nb,
                    )
                else:
                    nc.vector.tensor_scalar(
                        out=y_sb[:, qb, :], in0=out_sb[:, qb, :],
                        scalar1=mvm[:, ql, 0:1], scalar2=rstd,
                        op0=mybir.AluOpType.subtract, op1=mybir.AluOpType.mult,
                    )
                if split_y and ql % 2 == 1:
                    nc.sync.dma_start(
                        out=bass.AP(tensor=y.tensor,
                                    offset=y.offset + 128 * (qb - 1) * EC,
                                    ap=[[EC, 128], [128 * EC, 2], [1, EC]]),
                        in_=y_sb[:, qb - 1:qb + 1, :])

        # ---- main schedule ----------------------------------------------
        # PV trails scores by one head-pair so exp latency never stalls PE;
        # pv steps are interleaved between score kp-steps to avoid PSUM WAR
        # stalls (each stall also costs ~1.2us of PE p-state ramp).
        stats0 = small.tile([128, 4, 2, 6], F32, tag="stats", name="stats0")
        stats1 = small.tile([128, 4, 2, 6], F32, tag="stats", name="stats1")
        mvloc = [small.tile([128, 4, 2], F32, tag="mvloc", name=f"mvloc{q}")
                 for q in range(2)]

        emit_kq0()
        e00 = new_epair(0, 0)
        for _ in gen_scores_exp(0, 0, e00):
            pass
        emit_kq(1)

        # sc(0,1)/sc(1,1) run early (inside chunk 0) so the chunk-1 stretch
        # has extra PV blocks of PE work to cover its exp stream; V and the
        # pv stream trail one round so PE filler lasts to the end. Each pv
        # consumes the oldest pending e-pair (lag <= 3 pairs, epool bufs=8).
        import itertools
        e10 = new_epair(1, 0)
        interleave(gen_scores_exp(1, 0, e10), gen_v())
        emit_kq(2)
        e01 = new_epair(0, 1)
        interleave(gen_scores_exp(0, 1, e01), gen_pv(0, 0, e00))
        emit_kq(3)
        e20 = new_epair(2, 0)
        interleave(gen_scores_exp(2, 0, e20), gen_pv(1, 0, e10, stats0))
        e11 = new_epair(1, 1)
        interleave(gen_scores_exp(1, 1, e11), gen_pv(0, 1, e01))
        e30 = new_epair(3, 0)
        interleave(gen_scores_exp(3, 0, e30), gen_pv(2, 0, e20))
        e21 = new_epair(2, 1)
        interleave(gen_scores_exp(2, 1, e21), gen_pv(3, 0, e30, stats0, tail=True))

        allst0 = emit_cc(0, stats0)

        e31 = new_epair(3, 1)
        interleave(gen_scores_exp(3, 1, e31),
                   itertools.chain(gen_pv(1, 1, e11, stats1),
                                   gen_pv(2, 1, e21)), ratio=4)

        # Last head-pair: q-block-major; its tail partial bn_stats fires as
        # each q-block completes.
        for _ in gen_pv(3, 1, e31, stats1, tail=True, act=True):
            pass
        allst1 = emit_cc(1, stats1)

        # chunk-0 LN + output write hide under the final collective
        emit_ln_chunk(0, allst0, split_y=False)
        nc.sync.dma_start(
            out=bass.AP(tensor=y.tensor, offset=y.offset,
                        ap=[[EC, 128], [128 * EC, 4], [1, EC]]),
            in_=y_sb[:, 0:4, :],
        )
        emit_ln_chunk(1, allst1, split_y=True)


def kernel(x, Wq, Wk, Wv, bias_table, ln_gamma, ln_beta):
    x = np.asarray(x, np.float32)
    WqT = np.asarray(Wq, np.float32).T          # [E, E]: [in e, out e']
    WkT = np.asarray(Wk, np.float32).T
    WvT = np.asarray(Wv, np.float32).T
    tblT = np.asarray(bias_table, np.float32).T  # [H, 2S-1]
    g = np.asarray(ln_gamma, np.float32)
    bta = np.asarray(ln_beta, np.float32)

    if "nc" not in _cache:
        _cache["nc"] = _build_nc()
    nc = _cache["nc"]

    bf = ml_dtypes.bfloat16
    # xT pretiled: xT_t[b, p, k, s] = x[b, s, 128k+p]
    xT_t = np.ascontiguousarray(
        x.transpose(0, 2, 1).reshape(B, NK, 128, S).transpose(0, 2, 1, 3)
    ).astype(bf)
    # weights pretiled per core-half: w_t[p, k, c] = W^T[128k+p, col0+c]
    def wtile(WT, hg, width):  # [128, NK, width-block layout]
        Wc = WT[:, EC * hg: EC * (hg + 1)]       # [E, EC]
        return Wc.reshape(NK, 128, EC).transpose(1, 0, 2).astype(bf)

    # Toeplitz windows: tbl_t[h, p, u] = tblT[h, u - p + 127]
    p_i = np.arange(128)[:, None]
    u_i = np.arange(TW)[None, :]
    idx = 127 - p_i + u_i                        # in [0, 2046]
    tbl_all = np.ascontiguousarray(tblT[:, idx]).astype(bf)

    in_maps = []
    for c in range(8):
        b, hg = c // 2, c % 2
        wk_t = wtile(WkT, hg, 128)               # [128, NK, EC]
        wq_t = wtile(WqT, hg, 128)
        # wkq[p, j, k, 128]: j=0..3 wk m-slices, j=4..7 wq m-slices
        wkq_t = np.empty((128, 8, NK, 128), np.float32)
        for m in range(4):
            wkq_t[:, m] = wk_t[:, :, 128 * m:128 * (m + 1)]
            wkq_t[:, 4 + m] = wq_t[:, :, 128 * m:128 * (m + 1)]
        in_maps.append({
            "xT": xT_t[b],
            "wkq": np.ascontiguousarray(wkq_t).astype(bf),
            "wv": np.ascontiguousarray(wtile(WvT, hg, EC)),
            "tbl": np.ascontiguousarray(tbl_all[HC * hg: HC * (hg + 1)]),
        })

    res = run_bass_kernel_spmd(nc, in_maps, core_ids=list(range(8)))
    _cache["last_results"] = res

    out = np.empty((B, S, E), np.float32)
    for c in range(8):
        b, hg = c // 2, c % 2
        out[b, :, EC * hg: EC * (hg + 1)] = res.results[c]["y"]
    # gamma/beta are ones/zeros in this problem; apply on host if not.
    if not (np.all(g == 1.0) and np.all(bta == 0.0)):
        out = out * g + bta
    return out
